# revision 26
# baseline (speedup 1.0000x reference)
"""Trainium2 Bass kernel for nn_DenseExpert (soft-gated mixture of dense experts).

Math:  out[b,u] = sum_e gate[b,e] * (x[b,:] @ alpha[e]) [u] + (gate @ beta)[b,u]

Strategy (pure data parallel over batch, 8 cores). Per 512-row chunk per core:
  1. Host pre-casts x/gate to fp16 and pre-rotates layouts so every DMA is
     contiguous per partition; constants are packed so the preamble is a few
     large DMAs issued from different engines in parallel, ordered so the
     tensors gating the first chunk arrive first.
  2. Block-diag gate tensor dg[p, t, c, e] = gate[b(p,t),e] * [c == p%64]
     built in ONE fp16 tensor_tensor per chunk (layout chosen so all
     innermost strides are 1 -> DVE 2x mode); zero pattern from a host
     constant idrep2.
  3. zT ("scaled x transpose") via 8 row-tiled K=64 PE matmuls per chunk --
     pairs (l=0,1) run concurrently in disjoint row groups.
  4. PSUM->SBUF gather copies (fp32->fp16) write zT[i, t, l, e, c]; 1 tile
     on DVE / 3 on ACT per chunk.
  5. PE accumulates out.T[u, b] = sum_e alphaT_e.T @ zT_e + beta.T @ gateT
     (9 matmuls, one PSUM accumulation group).
  6. out.T copied to SBUF as fp16 and DMA'd to DRAM in [U, B] layout; host
     transposes + upcasts when assembling the full result.
  Dummy matmuls on zeroed SBUF warm the PE clock (HAM) during the preamble
  DMA wait.
"""

import dataclasses
from contextlib import ExitStack

import numpy as np

import concourse.bacc as bacc
import concourse.tile as tile
import concourse.mybir as mybir
from concourse.bass_utils import run_bass_kernel_spmd

F32 = mybir.dt.float32
F16 = mybir.dt.float16

B, E, I, U = 65536, 8, 128, 128
NCORES = 8
BLOC = B // NCORES        # 8192 batch rows per core
CHUNK = 512               # batch rows per pipeline chunk
NCHUNK = BLOC // CHUNK    # 16
TPC = CHUNK // 128        # 128-row tiles per chunk
KB = 64                   # contraction block for the diag trick
L = 128 // KB             # row-tiled matmuls per 128-row tile

# dpack[p, :] = [grot[p] (NCHUNK*TPC*E) | idrep2[p] (KB*E)]  -- gates chunk 0
DPACK_W = NCHUNK * TPC * E + KB * E
OFF_GROT = 0
OFF_IDREP = NCHUNK * TPC * E
GPACK_W = BLOC + U        # gateT row + beta row
WARMUP_MM = 6


def _build():
    nc = bacc.Bacc("TRN2", target_bir_lowering=False, debug=False)

    dpack = nc.dram_tensor("dpack", [128, DPACK_W], F16, kind="ExternalInput").ap()
    alphaT = nc.dram_tensor("alphaT", [128, E * U], F16, kind="ExternalInput").ap()
    gpack = nc.dram_tensor("gpack", [E, GPACK_W], F16, kind="ExternalInput").ap()
    xrot = nc.dram_tensor("xrot", [128, NCHUNK, TPC, I], F16, kind="ExternalInput").ap()
    outT = nc.dram_tensor("outT", [U, BLOC], F16, kind="ExternalOutput").ap()

    with tile.TileContext(nc) as tc, ExitStack() as ctx:
        const = ctx.enter_context(tc.tile_pool(name="const", bufs=1))
        xp = ctx.enter_context(tc.tile_pool(name="xp", bufs=4))
        dgp = ctx.enter_context(tc.tile_pool(name="dgp", bufs=3))
        ztp = ctx.enter_context(tc.tile_pool(name="ztp", bufs=3))
        op = ctx.enter_context(tc.tile_pool(name="op", bufs=3))
        ps_yt = ctx.enter_context(tc.tile_pool(name="ps_yt", bufs=3, space="PSUM"))
        ps_ot = ctx.enter_context(tc.tile_pool(name="ps_ot", bufs=2, space="PSUM"))

        # --- PE warmup fodder: zeroed SBUF tile, matmuls into a junk PSUM tile
        wz = const.tile([128, CHUNK], F16, tag="wz")
        nc.gpsimd.memset(wz[:], 0.0)

        # --- constants: dpack (sync, first: gates chunk-0 diag), gpack then
        # alphaT on gpsimd in parallel
        dpack_h = const.tile([128, DPACK_W], F16, tag="dpackh")
        nc.sync.dma_start(dpack_h[:], dpack)
        gpack_h = const.tile([E, GPACK_W], F16, tag="gpackh")
        nc.gpsimd.dma_start(gpack_h[:], gpack)
        alpha_sb = const.tile([128, E * U], F16, tag="alphah")
        nc.gpsimd.dma_start(alpha_sb[:], alphaT)

        def alpha_view(e):
            return dataclasses.replace(
                alpha_sb[:], ap=[[E * U, 128], [1, U]], offset=e * U
            )

        def grot_view(c):
            return dataclasses.replace(
                dpack_h[:],
                ap=[[DPACK_W, 128], [E, TPC], [0, KB], [1, E]],
                offset=OFF_GROT + c * TPC * E,
            )

        id_view = dataclasses.replace(
            dpack_h[:],
            ap=[[DPACK_W, 128], [0, TPC], [E, KB], [1, E]],
            offset=OFF_IDREP,
        )

        beta_view = dataclasses.replace(
            gpack_h[:], ap=[[GPACK_W, E], [1, U]], offset=BLOC
        )

        def gateT_view(c):
            return dataclasses.replace(
                gpack_h[:], ap=[[GPACK_W, E], [1, CHUNK]], offset=c * CHUNK
            )

        # --- PE warmup (runs during const/x DMA wait; inputs only need memset)
        wps = ps_ot.tile([128, CHUNK], F32, tag="ot")
        for _ in range(WARMUP_MM):
            nc.tensor.matmul(wps[:], wz[:, :128], wz[:], start=True, stop=True)

        def emit_front(c):
            x_h = xp.tile([128, TPC, I], F16, tag="xh")
            if c <= 1:
                nc.scalar.dma_start(x_h[:], xrot[:, c, :, :])
            else:
                nc.sync.dma_start(x_h[:], xrot[:, c, :, :])

            # dg[p, t, c, e] = idrep2[p, c, e] * gate[b(p,t), e]
            dg = dgp.tile([128, TPC, KB, E], F16, tag="dg")
            nc.vector.tensor_tensor(
                dg[:], id_view, grot_view(c), op=mybir.AluOpType.mult
            )

            zT = ztp.tile([128, TPC, L, KB, E], F16, tag="zT")
            for t in range(TPC):
                yt = ps_yt.tile([128, L, KB, E], F32, tag="yt")
                for l in range(L):
                    nc.tensor.matmul(
                        yt[:, l, :, :],
                        x_h[l * KB : (l + 1) * KB, t, :],
                        dg[l * KB : (l + 1) * KB, t, :, :],
                        start=True,
                        stop=True,
                    )
                # gather: zT[i, t, l, c, e] = yt[i, l, c, e]  (fp32->fp16,
                # dense source order; the (c,e)->(e,c) reorder lives in the
                # stage-2 rhs access pattern instead)
                dst = zT[:, t, :, :, :]
                if t == 0:
                    nc.vector.tensor_copy(dst, yt[:])
                else:
                    nc.scalar.copy(dst, yt[:])
            return zT

        def emit_back(c, zT):
            row0 = c * CHUNK
            ot = ps_ot.tile([128, CHUNK], F32, tag="ot")
            for e in range(E):
                rhs = dataclasses.replace(
                    zT[:],
                    ap=[[TPC * L * E * KB, 128], [L * E * KB, TPC], [E * KB, L], [E, KB]],
                    offset=e,
                )
                nc.tensor.matmul(
                    ot[:], alpha_view(e), rhs, start=(e == 0), stop=False
                )
            nc.tensor.matmul(ot[:], beta_view, gateT_view(c), start=False, stop=True)

            o16 = op.tile([128, CHUNK], F16, tag="o16")
            if c == NCHUNK - 1:
                # tail: split copy across engines + 2 DMAs to shorten drain
                nc.vector.tensor_copy(o16[:, : CHUNK // 2], ot[:, : CHUNK // 2])
                nc.scalar.copy(o16[:, CHUNK // 2 :], ot[:, CHUNK // 2 :])
                nc.sync.dma_start(
                    outT[:, row0 : row0 + CHUNK // 2], o16[:, : CHUNK // 2]
                )
                nc.sync.dma_start(
                    outT[:, row0 + CHUNK // 2 : row0 + CHUNK], o16[:, CHUNK // 2 :]
                )
            else:
                nc.vector.tensor_copy(o16[:], ot[:])
                nc.sync.dma_start(outT[:, row0 : row0 + CHUNK], o16[:])

        pending = None
        for c in range(NCHUNK):
            front = emit_front(c)
            if pending is not None:
                emit_back(c - 1, pending)
            pending = front
        emit_back(NCHUNK - 1, pending)

    nc.compile()
    return nc


_NC_CACHE = None


def _make_idrep2():
    idrep2 = np.zeros((128, KB, E), np.float16)
    for p in range(128):
        idrep2[p, p % KB, :] = 1.0
    return idrep2


def make_in_maps(x, gate_perc, alpha, beta):
    x16 = np.asarray(x, dtype=np.float16)
    g16 = np.asarray(gate_perc, dtype=np.float16)
    alphaT = (
        np.ascontiguousarray(np.asarray(alpha, dtype=np.float32).transpose(1, 0, 2))
        .astype(np.float16)
        .reshape(128, E * U)
    )
    beta16 = np.asarray(beta, dtype=np.float16)
    idrep2 = _make_idrep2().reshape(128, KB * E)
    in_maps = []
    for cid in range(NCORES):
        sl = slice(cid * BLOC, (cid + 1) * BLOC)
        xs, gs = x16[sl], g16[sl]
        xrot = np.ascontiguousarray(
            xs.reshape(NCHUNK, TPC, 128, I).transpose(2, 0, 1, 3)
        )
        grot = (
            np.ascontiguousarray(gs.reshape(NCHUNK, TPC, 128, E).transpose(2, 0, 1, 3))
            .reshape(128, NCHUNK * TPC * E)
        )
        dpack = np.concatenate([grot, idrep2], axis=1)
        gpack = np.concatenate([np.ascontiguousarray(gs.T), beta16], axis=1)
        in_maps.append(
            {"dpack": dpack, "alphaT": alphaT, "gpack": gpack, "xrot": xrot}
        )
    return in_maps


def kernel(x, gate_perc, alpha, beta):
    global _NC_CACHE
    if _NC_CACHE is None:
        _NC_CACHE = _build()
    nc = _NC_CACHE

    in_maps = make_in_maps(x, gate_perc, alpha, beta)
    res = run_bass_kernel_spmd(nc, in_maps, list(range(NCORES))).results
    # per-core outputs are [U, BLOC] fp16; assemble, transpose, upcast on host
    full_T = np.concatenate([res[c]["outT"] for c in range(NCORES)], axis=1)
    return np.ascontiguousarray(full_T.T).astype(np.float32)


if __name__ == "__main__":
    rng = np.random.default_rng(0)
    x = rng.standard_normal((B, I)).astype(np.float32)
    g = rng.random((B, E)).astype(np.float32)
    g /= g.sum(-1, keepdims=True)
    al = (rng.standard_normal((E, I, U)) * 0.05).astype(np.float32)
    be = (rng.standard_normal((E, U)) * 0.05).astype(np.float32)
    got = kernel(x, g, al, be)
    ref = np.einsum("bi,eio->beo", x, al, optimize=True)
    ref = np.einsum("beo,be->bo", ref, g) + g @ be
    err = np.abs(got - ref)
    print("max abs err", err.max(), "rel", err.max() / np.abs(ref).max())


# revision 29
# speedup vs baseline: 1.9009x; 1.9009x over previous
"""Trainium2 Bass kernel for nn_DenseExpert (soft-gated mixture of dense experts).

Math:  out[b,u] = sum_e gate[b,e] * (x[b,:] @ alpha[e]) [u] + (gate @ beta)[b,u]

Strategy (pure data parallel over batch, 8 cores). Per 512-row chunk per core:
  1. Host pre-casts x/gate to fp16 and pre-rotates layouts so every DMA is
     contiguous per partition; constants are packed so the preamble is a few
     large DMAs issued from different engines in parallel, ordered so the
     tensors gating the first chunk arrive first.
  2. Block-diag gate tensor dg[p, t, c, e] = gate[b(p,t),e] * [c == p%64]
     built in ONE fp16 tensor_tensor per chunk (layout chosen so all
     innermost strides are 1 -> DVE 2x mode); zero pattern from a host
     constant idrep2.
  3. zT ("scaled x transpose") via 8 row-tiled K=64 PE matmuls per chunk --
     pairs (l=0,1) run concurrently in disjoint row groups.
  4. PSUM->SBUF gather copies (fp32->fp16) write zT[i, t, l, e, c]; 1 tile
     on DVE / 3 on ACT per chunk.
  5. PE accumulates out.T[u, b] = sum_e alphaT_e.T @ zT_e + beta.T @ gateT
     (9 matmuls, one PSUM accumulation group).
  6. out.T copied to SBUF as fp16 and DMA'd to DRAM in [U, B] layout; host
     transposes + upcasts when assembling the full result.
  Dummy matmuls on zeroed SBUF warm the PE clock (HAM) during the preamble
  DMA wait.
"""

import dataclasses
from contextlib import ExitStack

import numpy as np

import concourse.bacc as bacc
import concourse.tile as tile
import concourse.mybir as mybir
from concourse.bass_utils import run_bass_kernel_spmd

F32 = mybir.dt.float32
F16 = mybir.dt.float16

B, E, I, U = 65536, 8, 128, 128
NCORES = 8
BLOC = B // NCORES        # 8192 batch rows per core
CHUNK = 512               # batch rows per pipeline chunk
NCHUNK = BLOC // CHUNK    # 16
TPC = CHUNK // 128        # 128-row tiles per chunk
KB = 64                   # contraction block for the diag trick
L = 128 // KB             # row-tiled matmuls per 128-row tile

# dpack[p, :] = [grot[p] (NCHUNK*TPC*E) | idrep2[p] (KB*E)]  -- gates chunk 0
DPACK_W = NCHUNK * TPC * E + KB * E
OFF_GROT = 0
OFF_IDREP = NCHUNK * TPC * E
GPACK_W = BLOC + U        # gateT row + beta row
WARMUP_MM = 6


def _build():
    nc = bacc.Bacc("TRN2", target_bir_lowering=False, debug=False)

    dpack = nc.dram_tensor("dpack", [128, DPACK_W], F16, kind="ExternalInput").ap()
    alphaT = nc.dram_tensor("alphaT", [128, E * U], F16, kind="ExternalInput").ap()
    gpack = nc.dram_tensor("gpack", [E, GPACK_W], F16, kind="ExternalInput").ap()
    xrot = nc.dram_tensor("xrot", [128, NCHUNK, TPC, I], F16, kind="ExternalInput").ap()
    outT = nc.dram_tensor("outT", [U, BLOC], F16, kind="ExternalOutput").ap()

    with tile.TileContext(nc) as tc, ExitStack() as ctx:
        const = ctx.enter_context(tc.tile_pool(name="const", bufs=1))
        xp = ctx.enter_context(tc.tile_pool(name="xp", bufs=4))
        dgp = ctx.enter_context(tc.tile_pool(name="dgp", bufs=3))
        ztp = ctx.enter_context(tc.tile_pool(name="ztp", bufs=3))
        op = ctx.enter_context(tc.tile_pool(name="op", bufs=3))
        ps_yt = ctx.enter_context(tc.tile_pool(name="ps_yt", bufs=3, space="PSUM"))
        ps_ot = ctx.enter_context(tc.tile_pool(name="ps_ot", bufs=2, space="PSUM"))

        # --- PE warmup fodder: zeroed SBUF tile, matmuls into a junk PSUM tile
        wz = const.tile([128, CHUNK], F16, tag="wz")
        nc.gpsimd.memset(wz[:], 0.0)

        # --- constants: dpack (sync, first: gates chunk-0 diag), gpack then
        # alphaT on gpsimd in parallel
        dpack_h = const.tile([128, DPACK_W], F16, tag="dpackh")
        nc.sync.dma_start(dpack_h[:], dpack)
        gpack_h = const.tile([E, GPACK_W], F16, tag="gpackh")
        nc.gpsimd.dma_start(gpack_h[:], gpack)
        alpha_sb = const.tile([128, E * U], F16, tag="alphah")
        nc.gpsimd.dma_start(alpha_sb[:], alphaT)

        def alpha_view(e):
            return dataclasses.replace(
                alpha_sb[:], ap=[[E * U, 128], [1, U]], offset=e * U
            )

        def grot_view(c):
            return dataclasses.replace(
                dpack_h[:],
                ap=[[DPACK_W, 128], [E, TPC], [0, KB], [1, E]],
                offset=OFF_GROT + c * TPC * E,
            )

        id_view = dataclasses.replace(
            dpack_h[:],
            ap=[[DPACK_W, 128], [0, TPC], [E, KB], [1, E]],
            offset=OFF_IDREP,
        )

        beta_view = dataclasses.replace(
            gpack_h[:], ap=[[GPACK_W, E], [1, U]], offset=BLOC
        )

        def gateT_view(c):
            return dataclasses.replace(
                gpack_h[:], ap=[[GPACK_W, E], [1, CHUNK]], offset=c * CHUNK
            )

        # --- PE warmup (runs during const/x DMA wait; inputs only need memset)
        wps = ps_ot.tile([128, CHUNK], F32, tag="ot")
        for _ in range(WARMUP_MM):
            nc.tensor.matmul(wps[:], wz[:, :128], wz[:], start=True, stop=True)
        wz_ps = [wps]

        def emit_front(c):
            x_h = xp.tile([128, TPC, I], F16, tag="xh")
            if c <= 1:
                nc.scalar.dma_start(x_h[:], xrot[:, c, :, :])
            else:
                nc.sync.dma_start(x_h[:], xrot[:, c, :, :])

            # dg[p, t, c, e] = idrep2[p, c, e] * gate[b(p,t), e]
            dg = dgp.tile([128, TPC, KB, E], F16, tag="dg")
            nc.vector.tensor_tensor(
                dg[:], id_view, grot_view(c), op=mybir.AluOpType.mult
            )

            zT = ztp.tile([128, TPC, L, E, KB], F16, tag="zT")
            for t in range(TPC):
                yt = ps_yt.tile([128, L, KB, E], F32, tag="yt")
                for l in range(L):
                    nc.tensor.matmul(
                        yt[:, l, :, :],
                        x_h[l * KB : (l + 1) * KB, t, :],
                        dg[l * KB : (l + 1) * KB, t, :, :],
                        start=True,
                        stop=True,
                    )
                # gather: zT[i, t, l, e, c] = yt[i, l, c, e]  (fp32->fp16)
                src = dataclasses.replace(
                    yt[:],
                    ap=[[L * KB * E, 128], [KB * E, L], [1, E], [E, KB]],
                    offset=0,
                )
                dst = zT[:, t, :, :, :]
                if t == 0:
                    nc.vector.tensor_copy(dst, src)
                else:
                    nc.scalar.copy(dst, src)
            if c <= 1:
                # keep the PE clock (HAM) warm through pipeline-fill stalls
                for _ in range(3 - c):
                    nc.tensor.matmul(
                        wz_ps[0][:], wz[:, :128], wz[:], start=True, stop=True
                    )
            return zT

        def emit_back(c, zT):
            row0 = c * CHUNK
            ot = ps_ot.tile([128, CHUNK], F32, tag="ot")
            for e in range(E):
                rhs = dataclasses.replace(
                    zT[:],
                    ap=[[TPC * L * E * KB, 128], [L * E * KB, TPC], [E * KB, L], [1, KB]],
                    offset=e * KB,
                )
                nc.tensor.matmul(
                    ot[:], alpha_view(e), rhs, start=(e == 0), stop=False
                )
            nc.tensor.matmul(ot[:], beta_view, gateT_view(c), start=False, stop=True)

            o16 = op.tile([128, CHUNK], F16, tag="o16")
            if c == NCHUNK - 1:
                # tail: split copy across engines + 2 DMAs to shorten drain
                nc.vector.tensor_copy(o16[:, : CHUNK // 2], ot[:, : CHUNK // 2])
                nc.scalar.copy(o16[:, CHUNK // 2 :], ot[:, CHUNK // 2 :])
                nc.sync.dma_start(
                    outT[:, row0 : row0 + CHUNK // 2], o16[:, : CHUNK // 2]
                )
                nc.sync.dma_start(
                    outT[:, row0 + CHUNK // 2 : row0 + CHUNK], o16[:, CHUNK // 2 :]
                )
            else:
                nc.vector.tensor_copy(o16[:], ot[:])
                nc.sync.dma_start(outT[:, row0 : row0 + CHUNK], o16[:])

        pending = None
        for c in range(NCHUNK):
            front = emit_front(c)
            if pending is not None:
                emit_back(c - 1, pending)
            pending = front
        emit_back(NCHUNK - 1, pending)

    nc.compile()
    return nc


_NC_CACHE = None


def _make_idrep2():
    idrep2 = np.zeros((128, KB, E), np.float16)
    for p in range(128):
        idrep2[p, p % KB, :] = 1.0
    return idrep2


def make_in_maps(x, gate_perc, alpha, beta):
    x16 = np.asarray(x, dtype=np.float16)
    g16 = np.asarray(gate_perc, dtype=np.float16)
    alphaT = (
        np.ascontiguousarray(np.asarray(alpha, dtype=np.float32).transpose(1, 0, 2))
        .astype(np.float16)
        .reshape(128, E * U)
    )
    beta16 = np.asarray(beta, dtype=np.float16)
    idrep2 = _make_idrep2().reshape(128, KB * E)
    in_maps = []
    for cid in range(NCORES):
        sl = slice(cid * BLOC, (cid + 1) * BLOC)
        xs, gs = x16[sl], g16[sl]
        xrot = np.ascontiguousarray(
            xs.reshape(NCHUNK, TPC, 128, I).transpose(2, 0, 1, 3)
        )
        grot = (
            np.ascontiguousarray(gs.reshape(NCHUNK, TPC, 128, E).transpose(2, 0, 1, 3))
            .reshape(128, NCHUNK * TPC * E)
        )
        dpack = np.concatenate([grot, idrep2], axis=1)
        gpack = np.concatenate([np.ascontiguousarray(gs.T), beta16], axis=1)
        in_maps.append(
            {"dpack": dpack, "alphaT": alphaT, "gpack": gpack, "xrot": xrot}
        )
    return in_maps


def kernel(x, gate_perc, alpha, beta):
    global _NC_CACHE
    if _NC_CACHE is None:
        _NC_CACHE = _build()
    nc = _NC_CACHE

    in_maps = make_in_maps(x, gate_perc, alpha, beta)
    res = run_bass_kernel_spmd(nc, in_maps, list(range(NCORES))).results
    # per-core outputs are [U, BLOC] fp16; assemble, transpose, upcast on host
    full_T = np.concatenate([res[c]["outT"] for c in range(NCORES)], axis=1)
    return np.ascontiguousarray(full_T.T).astype(np.float32)


if __name__ == "__main__":
    rng = np.random.default_rng(0)
    x = rng.standard_normal((B, I)).astype(np.float32)
    g = rng.random((B, E)).astype(np.float32)
    g /= g.sum(-1, keepdims=True)
    al = (rng.standard_normal((E, I, U)) * 0.05).astype(np.float32)
    be = (rng.standard_normal((E, U)) * 0.05).astype(np.float32)
    got = kernel(x, g, al, be)
    ref = np.einsum("bi,eio->beo", x, al, optimize=True)
    ref = np.einsum("beo,be->bo", ref, g) + g @ be
    err = np.abs(got - ref)
    print("max abs err", err.max(), "rel", err.max() / np.abs(ref).max())


# revision 31
# speedup vs baseline: 2.1867x; 1.1503x over previous
"""Trainium2 Bass kernel for nn_DenseExpert (soft-gated mixture of dense experts).

Math:  out[b,u] = sum_e gate[b,e] * (x[b,:] @ alpha[e]) [u] + (gate @ beta)[b,u]

Strategy (pure data parallel over batch, 8 cores). Per 512-row chunk per core:
  1. Host pre-casts x/gate to fp16 and pre-rotates layouts so every DMA is
     contiguous per partition; constants are packed so the preamble is a few
     large DMAs issued from different engines in parallel, ordered so the
     tensors gating the first chunk arrive first.
  2. Block-diag gate tensor dg[p, t, c, e] = gate[b(p,t),e] * [c == p%64]
     built in ONE fp16 tensor_tensor per chunk (layout chosen so all
     innermost strides are 1 -> DVE 2x mode); zero pattern from a host
     constant idrep2.
  3. zT ("scaled x transpose") via 8 row-tiled K=64 PE matmuls per chunk --
     pairs (l=0,1) run concurrently in disjoint row groups.
  4. PSUM->SBUF gather copies (fp32->fp16) write zT[i, t, l, e, c]; 1 tile
     on DVE / 3 on ACT per chunk.
  5. PE accumulates out.T[u, b] = sum_e alphaT_e.T @ zT_e + beta.T @ gateT
     (9 matmuls, one PSUM accumulation group).
  6. out.T copied to SBUF as fp16 and DMA'd to DRAM in [U, B] layout; host
     transposes + upcasts when assembling the full result.
  Dummy matmuls on zeroed SBUF warm the PE clock (HAM) during the preamble
  DMA wait.
"""

import dataclasses
from contextlib import ExitStack

import numpy as np

import concourse.bacc as bacc
import concourse.tile as tile
import concourse.mybir as mybir
from concourse.bass_utils import run_bass_kernel_spmd

F32 = mybir.dt.float32
F16 = mybir.dt.float16

B, E, I, U = 65536, 8, 128, 128
NCORES = 8
BLOC = B // NCORES        # 8192 batch rows per core
CHUNK = 512               # batch rows per pipeline chunk
NCHUNK = BLOC // CHUNK    # 16
TPC = CHUNK // 128        # 128-row tiles per chunk
KB = 64                   # contraction block for the diag trick
L = 128 // KB             # row-tiled matmuls per 128-row tile

# dpack[p, :] = [grot[p] (NCHUNK*TPC*E) | idrep2[p] (KB*E)]  -- gates chunk 0
DPACK_W = NCHUNK * TPC * E + KB * E
OFF_GROT = 0
OFF_IDREP = NCHUNK * TPC * E
GPACK_W = BLOC + U        # gateT row + beta row
WARMUP_MM = 6


def _build():
    nc = bacc.Bacc("TRN2", target_bir_lowering=False, debug=False)

    dpack = nc.dram_tensor("dpack", [128, DPACK_W], F16, kind="ExternalInput").ap()
    alphaT = nc.dram_tensor("alphaT", [128, E * U], F16, kind="ExternalInput").ap()
    gpack = nc.dram_tensor("gpack", [E, GPACK_W], F16, kind="ExternalInput").ap()
    xrot = nc.dram_tensor("xrot", [128, NCHUNK, TPC, I], F16, kind="ExternalInput").ap()
    outT = nc.dram_tensor("outT", [U, BLOC], F16, kind="ExternalOutput").ap()

    with tile.TileContext(nc) as tc, ExitStack() as ctx:
        const = ctx.enter_context(tc.tile_pool(name="const", bufs=1))
        xp = ctx.enter_context(tc.tile_pool(name="xp", bufs=4))
        dgp = ctx.enter_context(tc.tile_pool(name="dgp", bufs=3))
        ztp = ctx.enter_context(tc.tile_pool(name="ztp", bufs=3))
        op = ctx.enter_context(tc.tile_pool(name="op", bufs=3))
        ps_yt = ctx.enter_context(tc.tile_pool(name="ps_yt", bufs=3, space="PSUM"))
        ps_ot = ctx.enter_context(tc.tile_pool(name="ps_ot", bufs=2, space="PSUM"))

        # --- PE warmup fodder: zeroed SBUF tile, matmuls into a junk PSUM tile
        wz = const.tile([128, CHUNK], F16, tag="wz")
        nc.gpsimd.memset(wz[:], 0.0)

        # --- constants: dpack (sync, first: gates chunk-0 diag), gpack then
        # alphaT on gpsimd in parallel
        dpack_h = const.tile([128, DPACK_W], F16, tag="dpackh")
        nc.sync.dma_start(dpack_h[:], dpack)
        gpack_h = const.tile([E, GPACK_W], F16, tag="gpackh")
        nc.gpsimd.dma_start(gpack_h[:], gpack)
        alpha_sb = const.tile([128, E * U], F16, tag="alphah")
        nc.gpsimd.dma_start(alpha_sb[:], alphaT)

        def alpha_view(e):
            return dataclasses.replace(
                alpha_sb[:], ap=[[E * U, 128], [1, U]], offset=e * U
            )

        def grot_view(c):
            return dataclasses.replace(
                dpack_h[:],
                ap=[[DPACK_W, 128], [E, TPC], [0, KB], [1, E]],
                offset=OFF_GROT + c * TPC * E,
            )

        id_view = dataclasses.replace(
            dpack_h[:],
            ap=[[DPACK_W, 128], [0, TPC], [E, KB], [1, E]],
            offset=OFF_IDREP,
        )

        beta_view = dataclasses.replace(
            gpack_h[:], ap=[[GPACK_W, E], [1, U]], offset=BLOC
        )

        def gateT_view(c):
            return dataclasses.replace(
                gpack_h[:], ap=[[GPACK_W, E], [1, CHUNK]], offset=c * CHUNK
            )

        # --- PE warmup (runs during const/x DMA wait; inputs only need memset)
        wps = ps_ot.tile([128, CHUNK], F32, tag="ot")
        for _ in range(WARMUP_MM):
            nc.tensor.matmul(wps[:], wz[:, :128], wz[:], start=True, stop=True)

        def emit_front(c):
            x_h = xp.tile([128, TPC, I], F16, tag="xh")
            if c <= 1:
                nc.scalar.dma_start(x_h[:], xrot[:, c, :, :])
            else:
                nc.sync.dma_start(x_h[:], xrot[:, c, :, :])

            # dg[p, t, c, e] = idrep2[p, c, e] * gate[b(p,t), e]
            dg = dgp.tile([128, TPC, KB, E], F16, tag="dg")
            nc.vector.tensor_tensor(
                dg[:], id_view, grot_view(c), op=mybir.AluOpType.mult
            )

            zT = ztp.tile([128, TPC, L, E, KB], F16, tag="zT")
            for t in range(TPC):
                yt = ps_yt.tile([128, L, KB, E], F32, tag="yt")
                for l in range(L):
                    nc.tensor.matmul(
                        yt[:, l, :, :],
                        x_h[l * KB : (l + 1) * KB, t, :],
                        dg[l * KB : (l + 1) * KB, t, :, :],
                        start=True,
                        stop=True,
                    )
                # gather: zT[i, t, l, e, c] = yt[i, l, c, e]  (fp32->fp16)
                src = dataclasses.replace(
                    yt[:],
                    ap=[[L * KB * E, 128], [KB * E, L], [1, E], [E, KB]],
                    offset=0,
                )
                dst = zT[:, t, :, :, :]
                if t == 0:
                    nc.vector.tensor_copy(dst, src)
                else:
                    nc.scalar.copy(dst, src)
            return zT

        def emit_back(c, zT):
            row0 = c * CHUNK
            ot = ps_ot.tile([128, CHUNK], F32, tag="ot")
            for e in range(E):
                rhs = dataclasses.replace(
                    zT[:],
                    ap=[[TPC * L * E * KB, 128], [L * E * KB, TPC], [E * KB, L], [1, KB]],
                    offset=e * KB,
                )
                nc.tensor.matmul(
                    ot[:], alpha_view(e), rhs, start=(e == 0), stop=False
                )
            nc.tensor.matmul(ot[:], beta_view, gateT_view(c), start=False, stop=True)

            o16 = op.tile([128, CHUNK], F16, tag="o16")
            if c == NCHUNK - 1:
                # tail: split copy across engines + 2 DMAs to shorten drain
                nc.vector.tensor_copy(o16[:, : CHUNK // 2], ot[:, : CHUNK // 2])
                nc.scalar.copy(o16[:, CHUNK // 2 :], ot[:, CHUNK // 2 :])
                nc.sync.dma_start(
                    outT[:, row0 : row0 + CHUNK // 2], o16[:, : CHUNK // 2]
                )
                nc.sync.dma_start(
                    outT[:, row0 + CHUNK // 2 : row0 + CHUNK], o16[:, CHUNK // 2 :]
                )
            else:
                nc.vector.tensor_copy(o16[:], ot[:])
                nc.sync.dma_start(outT[:, row0 : row0 + CHUNK], o16[:])

        pending = None
        for c in range(NCHUNK):
            front = emit_front(c)
            if pending is not None:
                emit_back(c - 1, pending)
            pending = front
        emit_back(NCHUNK - 1, pending)

    nc.compile()
    return nc


_NC_CACHE = None


def _make_idrep2():
    idrep2 = np.zeros((128, KB, E), np.float16)
    for p in range(128):
        idrep2[p, p % KB, :] = 1.0
    return idrep2


def make_in_maps(x, gate_perc, alpha, beta):
    x16 = np.asarray(x, dtype=np.float16)
    g16 = np.asarray(gate_perc, dtype=np.float16)
    alphaT = (
        np.ascontiguousarray(np.asarray(alpha, dtype=np.float32).transpose(1, 0, 2))
        .astype(np.float16)
        .reshape(128, E * U)
    )
    beta16 = np.asarray(beta, dtype=np.float16)
    idrep2 = _make_idrep2().reshape(128, KB * E)
    in_maps = []
    for cid in range(NCORES):
        sl = slice(cid * BLOC, (cid + 1) * BLOC)
        xs, gs = x16[sl], g16[sl]
        xrot = np.ascontiguousarray(
            xs.reshape(NCHUNK, TPC, 128, I).transpose(2, 0, 1, 3)
        )
        grot = (
            np.ascontiguousarray(gs.reshape(NCHUNK, TPC, 128, E).transpose(2, 0, 1, 3))
            .reshape(128, NCHUNK * TPC * E)
        )
        dpack = np.concatenate([grot, idrep2], axis=1)
        gpack = np.concatenate([np.ascontiguousarray(gs.T), beta16], axis=1)
        in_maps.append(
            {"dpack": dpack, "alphaT": alphaT, "gpack": gpack, "xrot": xrot}
        )
    return in_maps


def kernel(x, gate_perc, alpha, beta):
    global _NC_CACHE
    if _NC_CACHE is None:
        _NC_CACHE = _build()
    nc = _NC_CACHE

    in_maps = make_in_maps(x, gate_perc, alpha, beta)
    res = run_bass_kernel_spmd(nc, in_maps, list(range(NCORES))).results
    # per-core outputs are [U, BLOC] fp16; assemble, transpose, upcast on host
    full_T = np.concatenate([res[c]["outT"] for c in range(NCORES)], axis=1)
    return np.ascontiguousarray(full_T.T).astype(np.float32)


if __name__ == "__main__":
    rng = np.random.default_rng(0)
    x = rng.standard_normal((B, I)).astype(np.float32)
    g = rng.random((B, E)).astype(np.float32)
    g /= g.sum(-1, keepdims=True)
    al = (rng.standard_normal((E, I, U)) * 0.05).astype(np.float32)
    be = (rng.standard_normal((E, U)) * 0.05).astype(np.float32)
    got = kernel(x, g, al, be)
    ref = np.einsum("bi,eio->beo", x, al, optimize=True)
    ref = np.einsum("beo,be->bo", ref, g) + g @ be
    err = np.abs(got - ref)
    print("max abs err", err.max(), "rel", err.max() / np.abs(ref).max())


# revision 32
# speedup vs baseline: 2.2425x; 1.0255x over previous
"""Trainium2 Bass kernel for nn_DenseExpert (soft-gated mixture of dense experts).

Math:  out[b,u] = sum_e gate[b,e] * (x[b,:] @ alpha[e]) [u] + (gate @ beta)[b,u]

Strategy (pure data parallel over batch, 8 cores). Per 512-row chunk per core:
  1. Host pre-casts x/gate to fp16 and pre-rotates layouts so every DMA is
     contiguous per partition; constants are packed so the preamble is a few
     large DMAs issued from different engines in parallel, ordered so the
     tensors gating the first chunk arrive first.
  2. Block-diag gate tensor dg[p, t, c, e] = gate[b(p,t),e] * [c == p%64]
     built in ONE fp16 tensor_tensor per chunk (layout chosen so all
     innermost strides are 1 -> DVE 2x mode); zero pattern from a host
     constant idrep2.
  3. zT ("scaled x transpose") via 8 row-tiled K=64 PE matmuls per chunk --
     pairs (l=0,1) run concurrently in disjoint row groups.
  4. PSUM->SBUF gather copies (fp32->fp16) write zT[i, t, l, e, c]; 1 tile
     on DVE / 3 on ACT per chunk.
  5. PE accumulates out.T[u, b] = sum_e alphaT_e.T @ zT_e + beta.T @ gateT
     (9 matmuls, one PSUM accumulation group).
  6. out.T copied to SBUF as fp16 and DMA'd to DRAM in [U, B] layout; host
     transposes + upcasts when assembling the full result.
  Dummy matmuls on zeroed SBUF warm the PE clock (HAM) during the preamble
  DMA wait.
"""

import dataclasses
from contextlib import ExitStack

import numpy as np

import concourse.bacc as bacc
import concourse.tile as tile
import concourse.mybir as mybir
from concourse.bass_utils import run_bass_kernel_spmd

F32 = mybir.dt.float32
F16 = mybir.dt.float16

B, E, I, U = 65536, 8, 128, 128
NCORES = 8
BLOC = B // NCORES        # 8192 batch rows per core
CHUNK = 512               # batch rows per pipeline chunk
NCHUNK = BLOC // CHUNK    # 16
TPC = CHUNK // 128        # 128-row tiles per chunk
KB = 64                   # contraction block for the diag trick
L = 128 // KB             # row-tiled matmuls per 128-row tile

# dpack[p, :] = [grot[p] (NCHUNK*TPC*E) | idrep2[p] (KB*E)]  -- gates chunk 0
DPACK_W = NCHUNK * TPC * E + KB * E
OFF_GROT = 0
OFF_IDREP = NCHUNK * TPC * E
GPACK_W = BLOC + U        # gateT row + beta row
WARMUP_MM = 6


def _build():
    nc = bacc.Bacc("TRN2", target_bir_lowering=False, debug=False)

    dpack = nc.dram_tensor("dpack", [128, DPACK_W], F16, kind="ExternalInput").ap()
    alphaT = nc.dram_tensor("alphaT", [128, E * U], F16, kind="ExternalInput").ap()
    gpack = nc.dram_tensor("gpack", [E, GPACK_W], F16, kind="ExternalInput").ap()
    xrot = nc.dram_tensor("xrot", [128, NCHUNK, TPC, I], F16, kind="ExternalInput").ap()
    outT = nc.dram_tensor("outT", [U, BLOC], F16, kind="ExternalOutput").ap()

    with tile.TileContext(nc) as tc, ExitStack() as ctx:
        const = ctx.enter_context(tc.tile_pool(name="const", bufs=1))
        xp = ctx.enter_context(tc.tile_pool(name="xp", bufs=6))
        dgp = ctx.enter_context(tc.tile_pool(name="dgp", bufs=4))
        ztp = ctx.enter_context(tc.tile_pool(name="ztp", bufs=4))
        op = ctx.enter_context(tc.tile_pool(name="op", bufs=4))
        ps_yt = ctx.enter_context(tc.tile_pool(name="ps_yt", bufs=3, space="PSUM"))
        ps_ot = ctx.enter_context(tc.tile_pool(name="ps_ot", bufs=2, space="PSUM"))

        # --- PE warmup fodder: zeroed SBUF tile, matmuls into a junk PSUM tile
        wz = const.tile([128, CHUNK], F16, tag="wz")
        nc.gpsimd.memset(wz[:], 0.0)

        # --- constants: dpack (sync, first: gates chunk-0 diag), gpack then
        # alphaT on gpsimd in parallel
        dpack_h = const.tile([128, DPACK_W], F16, tag="dpackh")
        nc.sync.dma_start(dpack_h[:], dpack)
        gpack_h = const.tile([E, GPACK_W], F16, tag="gpackh")
        nc.gpsimd.dma_start(gpack_h[:], gpack)
        alpha_sb = const.tile([128, E * U], F16, tag="alphah")
        nc.gpsimd.dma_start(alpha_sb[:], alphaT)

        def alpha_view(e):
            return dataclasses.replace(
                alpha_sb[:], ap=[[E * U, 128], [1, U]], offset=e * U
            )

        def grot_view(c):
            return dataclasses.replace(
                dpack_h[:],
                ap=[[DPACK_W, 128], [E, TPC], [0, KB], [1, E]],
                offset=OFF_GROT + c * TPC * E,
            )

        id_view = dataclasses.replace(
            dpack_h[:],
            ap=[[DPACK_W, 128], [0, TPC], [E, KB], [1, E]],
            offset=OFF_IDREP,
        )

        beta_view = dataclasses.replace(
            gpack_h[:], ap=[[GPACK_W, E], [1, U]], offset=BLOC
        )

        def gateT_view(c):
            return dataclasses.replace(
                gpack_h[:], ap=[[GPACK_W, E], [1, CHUNK]], offset=c * CHUNK
            )

        # --- PE warmup (runs during const/x DMA wait; inputs only need memset)
        wps = ps_ot.tile([128, CHUNK], F32, tag="ot")
        for _ in range(WARMUP_MM):
            nc.tensor.matmul(wps[:], wz[:, :128], wz[:], start=True, stop=True)

        def emit_front(c):
            x_h = xp.tile([128, TPC, I], F16, tag="xh")
            if c <= 1:
                nc.scalar.dma_start(x_h[:], xrot[:, c, :, :])
            else:
                nc.sync.dma_start(x_h[:], xrot[:, c, :, :])

            # dg[p, t, c, e] = idrep2[p, c, e] * gate[b(p,t), e]
            dg = dgp.tile([128, TPC, KB, E], F16, tag="dg")
            nc.vector.tensor_tensor(
                dg[:], id_view, grot_view(c), op=mybir.AluOpType.mult
            )

            zT = ztp.tile([128, TPC, L, E, KB], F16, tag="zT")
            for t in range(TPC):
                yt = ps_yt.tile([128, L, KB, E], F32, tag="yt")
                for l in range(L):
                    nc.tensor.matmul(
                        yt[:, l, :, :],
                        x_h[l * KB : (l + 1) * KB, t, :],
                        dg[l * KB : (l + 1) * KB, t, :, :],
                        start=True,
                        stop=True,
                    )
                # gather: zT[i, t, l, e, c] = yt[i, l, c, e]  (fp32->fp16)
                src = dataclasses.replace(
                    yt[:],
                    ap=[[L * KB * E, 128], [KB * E, L], [1, E], [E, KB]],
                    offset=0,
                )
                dst = zT[:, t, :, :, :]
                if t == 0:
                    nc.vector.tensor_copy(dst, src)
                else:
                    nc.scalar.copy(dst, src)
            return zT

        def emit_back(c, zT):
            row0 = c * CHUNK
            ot = ps_ot.tile([128, CHUNK], F32, tag="ot")
            for e in range(E):
                rhs = dataclasses.replace(
                    zT[:],
                    ap=[[TPC * L * E * KB, 128], [L * E * KB, TPC], [E * KB, L], [1, KB]],
                    offset=e * KB,
                )
                nc.tensor.matmul(
                    ot[:], alpha_view(e), rhs, start=(e == 0), stop=False
                )
            nc.tensor.matmul(ot[:], beta_view, gateT_view(c), start=False, stop=True)

            o16 = op.tile([128, CHUNK], F16, tag="o16")
            if c == NCHUNK - 1:
                # tail: split copy across engines + 2 DMAs to shorten drain
                nc.vector.tensor_copy(o16[:, : CHUNK // 2], ot[:, : CHUNK // 2])
                nc.scalar.copy(o16[:, CHUNK // 2 :], ot[:, CHUNK // 2 :])
                nc.sync.dma_start(
                    outT[:, row0 : row0 + CHUNK // 2], o16[:, : CHUNK // 2]
                )
                nc.sync.dma_start(
                    outT[:, row0 + CHUNK // 2 : row0 + CHUNK], o16[:, CHUNK // 2 :]
                )
            else:
                nc.vector.tensor_copy(o16[:], ot[:])
                nc.sync.dma_start(outT[:, row0 : row0 + CHUNK], o16[:])

        pending = None
        for c in range(NCHUNK):
            front = emit_front(c)
            if pending is not None:
                emit_back(c - 1, pending)
            pending = front
        emit_back(NCHUNK - 1, pending)

    nc.compile()
    return nc


_NC_CACHE = None


def _make_idrep2():
    idrep2 = np.zeros((128, KB, E), np.float16)
    for p in range(128):
        idrep2[p, p % KB, :] = 1.0
    return idrep2


def make_in_maps(x, gate_perc, alpha, beta):
    x16 = np.asarray(x, dtype=np.float16)
    g16 = np.asarray(gate_perc, dtype=np.float16)
    alphaT = (
        np.ascontiguousarray(np.asarray(alpha, dtype=np.float32).transpose(1, 0, 2))
        .astype(np.float16)
        .reshape(128, E * U)
    )
    beta16 = np.asarray(beta, dtype=np.float16)
    idrep2 = _make_idrep2().reshape(128, KB * E)
    in_maps = []
    for cid in range(NCORES):
        sl = slice(cid * BLOC, (cid + 1) * BLOC)
        xs, gs = x16[sl], g16[sl]
        xrot = np.ascontiguousarray(
            xs.reshape(NCHUNK, TPC, 128, I).transpose(2, 0, 1, 3)
        )
        grot = (
            np.ascontiguousarray(gs.reshape(NCHUNK, TPC, 128, E).transpose(2, 0, 1, 3))
            .reshape(128, NCHUNK * TPC * E)
        )
        dpack = np.concatenate([grot, idrep2], axis=1)
        gpack = np.concatenate([np.ascontiguousarray(gs.T), beta16], axis=1)
        in_maps.append(
            {"dpack": dpack, "alphaT": alphaT, "gpack": gpack, "xrot": xrot}
        )
    return in_maps


def kernel(x, gate_perc, alpha, beta):
    global _NC_CACHE
    if _NC_CACHE is None:
        _NC_CACHE = _build()
    nc = _NC_CACHE

    in_maps = make_in_maps(x, gate_perc, alpha, beta)
    res = run_bass_kernel_spmd(nc, in_maps, list(range(NCORES))).results
    # per-core outputs are [U, BLOC] fp16; assemble, transpose, upcast on host
    full_T = np.concatenate([res[c]["outT"] for c in range(NCORES)], axis=1)
    return np.ascontiguousarray(full_T.T).astype(np.float32)


if __name__ == "__main__":
    rng = np.random.default_rng(0)
    x = rng.standard_normal((B, I)).astype(np.float32)
    g = rng.random((B, E)).astype(np.float32)
    g /= g.sum(-1, keepdims=True)
    al = (rng.standard_normal((E, I, U)) * 0.05).astype(np.float32)
    be = (rng.standard_normal((E, U)) * 0.05).astype(np.float32)
    got = kernel(x, g, al, be)
    ref = np.einsum("bi,eio->beo", x, al, optimize=True)
    ref = np.einsum("beo,be->bo", ref, g) + g @ be
    err = np.abs(got - ref)
    print("max abs err", err.max(), "rel", err.max() / np.abs(ref).max())
